# revision 20
# baseline (speedup 1.0000x reference)
"""Expert-parallel Gemma MoE kernel for 8 Trainium2 NeuronCores.

Strategy: one expert per core. Host gathers each expert's routed tokens
(padded to a fixed capacity), pre-transposes and casts to bf16; each core
runs the gated MLP (gate_up matmul -> gelu_tanh * up -> down matmul) on its
token slab in bf16 with fp32 PSUM accumulation; host scatter-adds the
weighted per-expert outputs back into the full [T, H] fp32 output.

All matmuls keep tokens on the moving (free) dimension so no on-device
transposes are needed: the host supplies X^T, W_gu^T and W_d^T and the
device produces y^T.
"""

import functools
import math

import numpy as np
import ml_dtypes

from concourse import bacc, bass, tile
from concourse import mybir

# Problem constants (nn_Gemma4TextExperts: Gemma-style MoE).
T = 2048      # tokens
H = 1024      # hidden
I = 2048      # intermediate
E = 8         # experts = cores
TOPK = 2

P = 128       # SBUF partitions
NMAX = 512    # max moving free dim per matmul (one PSUM bank of fp32)

BF16 = mybir.dt.bfloat16
F32 = mybir.dt.float32

KH = H // P       # 8  k-tiles for the H contraction
KI = I // P       # 16 k-tiles for the I contraction
MGU = 2 * I // P  # 32 output row tiles of gate_up (16 gate + 16 up)
MH = H // P       # 8  output row tiles of down


def _build_bass(nc_free: int, nchunks: int):
    """Build the single-core Bass program. nc_free = tokens per chunk,
    nchunks = number of token chunks; capacity = nc_free * nchunks."""
    cap = nc_free * nchunks
    # Bacc (not raw Bass): its compile() runs generate_event_semaphores,
    # which splits multi-sem sync waits that TRN2 instructions can't carry.
    nc = bacc.Bacc()

    n_gub = 2 * I // NMAX   # 8 column blocks of wgu (0..3 gate, 4..7 up)
    n_db = H // NMAX        # 2 column blocks of wd
    mg_per_b = NMAX // P    # 4 m-tiles per wgu block
    mh_per_b = NMAX // P    # 4 mh-tiles per wd block

    # DRAM layouts are partition-major per block so each block is ONE fully
    # contiguous DMA (the host pre-permutes; a block's [P, K, N] matches its
    # SBUF tile exactly).
    xt_d = nc.declare_dram_parameter("xt", [nchunks, P, KH, nc_free], BF16, isOutput=False)
    wgu_d = nc.declare_dram_parameter("wgu", [n_gub, P, KH, NMAX], BF16, isOutput=False)
    wd_d = nc.declare_dram_parameter("wd", [n_db, P, KI, NMAX], BF16, isOutput=False)
    yt_d = nc.declare_dram_parameter("yt", [MH, P, cap], F32, isOutput=True)

    with tile.TileContext(nc) as tc:
        with (
            tc.tile_pool(name="wpool", bufs=1) as wpool,
            tc.tile_pool(name="xpool", bufs=1) as xpool,
            tc.tile_pool(name="hpool", bufs=1) as hpool,
            tc.tile_pool(name="gpool", bufs=16) as gpool,
            tc.tile_pool(name="upool", bufs=16) as upool,
            tc.tile_pool(name="opool", bufs=8) as opool,
            tc.tile_pool(name="ppool", bufs=2, space=bass.MemorySpace.PSUM) as ppool,
        ):
            # Weights, loaded once. Per-block tiles so matmuls only wait on
            # the block they read. DMA issue order matters beyond deps: each
            # HW queue is in-order and consumers wait on a queue-sem
            # threshold, so anything enqueued before a tile's DMA delays
            # every consumer of that tile. Issue in consumption order:
            # xt slabs, then gate/up blocks interleaved, then down blocks.
            wgu_sb = [
                wpool.tile([P, KH, NMAX], BF16, tag=f"wgu{j}", name=f"wgu{j}")
                for j in range(n_gub)
            ]
            wd_sb = [
                wpool.tile([P, KI, NMAX], BF16, tag=f"wd{j}", name=f"wd{j}")
                for j in range(n_db)
            ]
            xt_tiles = [
                xpool.tile([P, KH, nc_free], BF16, tag=f"xt{cc}", name=f"xt{cc}")
                for cc in range(nchunks)
            ]

            # One contiguous ~1MB DMA per block. Token slabs ride the ACT
            # HWDGE ring (with the output stores); weights ride the SP ring.
            # Two rings double descriptor-gen throughput and keep stores
            # from queuing behind loads.
            for cc in range(nchunks):
                nc.scalar.dma_start(out=xt_tiles[cc][:, :, :], in_=xt_d[cc])
            gu_order = [j for pair in zip(range(n_gub // 2), range(n_gub // 2, n_gub))
                        for j in pair] if n_gub > 1 else list(range(n_gub))
            for j in gu_order:
                nc.sync.dma_start(out=wgu_sb[j][:, :, :], in_=wgu_d[j])
            for j in range(n_db):
                nc.sync.dma_start(out=wd_sb[j][:, :, :], in_=wd_d[j])

            for cc in range(nchunks):
                c0 = cc * nc_free
                xt_sb = xt_tiles[cc]

                # h^T tiles: [P, KI, nc_free] bf16 (the gelu(gate)*up result).
                h_sb = hpool.tile([P, KI, nc_free], BF16, tag="h")

                for m in range(KI):  # 16 (gate, up) row-tile pairs
                    jg, og = m // mg_per_b, (m % mg_per_b) * P
                    ju, ou = jg + n_gub // 2, og
                    pg = ppool.tile([P, nc_free], F32, tag="pg")
                    pu = ppool.tile([P, nc_free], F32, tag="pu")
                    for k in range(KH):
                        nc.tensor.matmul(
                            pg[:, :],
                            wgu_sb[jg][:, k, og:og + P],
                            xt_sb[:, k, :],
                            start=(k == 0),
                            stop=(k == KH - 1),
                        )
                    for k in range(KH):
                        nc.tensor.matmul(
                            pu[:, :],
                            wgu_sb[ju][:, k, ou:ou + P],
                            xt_sb[:, k, :],
                            start=(k == 0),
                            stop=(k == KH - 1),
                        )
                    g_sb = gpool.tile([P, nc_free], BF16, tag="g")
                    nc.scalar.activation(
                        g_sb[:, :], pg[:, :],
                        mybir.ActivationFunctionType.Gelu_apprx_tanh,
                    )
                    nc.vector.tensor_mul(h_sb[:, m, :], g_sb[:, :], pu[:, :])

                for mh in range(MH):  # 8 output row tiles
                    jd, od = mh // mh_per_b, (mh % mh_per_b) * P
                    py = ppool.tile([P, nc_free], F32, tag="py")
                    for k in range(KI):
                        nc.tensor.matmul(
                            py[:, :],
                            wd_sb[jd][:, k, od:od + P],
                            h_sb[:, k, :],
                            start=(k == 0),
                            stop=(k == KI - 1),
                        )
                    o_sb = opool.tile([P, nc_free], F32, tag="o")
                    nc.vector.tensor_copy(o_sb[:, :], py[:, :])
                    nc.scalar.dma_start(
                        out=yt_d[mh, :, c0:c0 + nc_free], in_=o_sb[:, :]
                    )

    nc.finalize()
    return nc


@functools.lru_cache(maxsize=4)
def _get_program(nc_free: int, nchunks: int):
    return _build_bass(nc_free, nchunks)


def prepare_in_maps(x, gup, dp, tok_lists, nc_free, nchunks):
    """Per-core input dicts in the partition-major block layouts the
    device program expects (see _build_bass)."""
    cap = nc_free * nchunks
    in_maps = []
    for c in range(len(tok_lists)):
        tl = tok_lists[c]
        xt = np.zeros((H, cap), ml_dtypes.bfloat16)
        if len(tl):
            xt[:, :len(tl)] = x[tl].T
        xt_b = np.ascontiguousarray(
            xt.reshape(KH, P, nchunks, nc_free).transpose(2, 1, 0, 3))
        wgu_b = np.ascontiguousarray(
            gup[c].T.astype(ml_dtypes.bfloat16)
            .reshape(KH, P, 2 * I // NMAX, NMAX).transpose(2, 1, 0, 3))
        wd_b = np.ascontiguousarray(
            dp[c].T.astype(ml_dtypes.bfloat16)
            .reshape(KI, P, H // NMAX, NMAX).transpose(2, 1, 0, 3))
        in_maps.append({"xt": xt_b, "wgu": wgu_b, "wd": wd_b})
    return in_maps


def kernel(hidden_states, top_k_index, top_k_weights, gate_up_proj, down_proj):
    from concourse.bass_utils import run_bass_kernel_spmd

    x = np.asarray(hidden_states, dtype=np.float32)
    idx = np.asarray(top_k_index)
    tkw = np.asarray(top_k_weights, dtype=np.float32)
    gup = np.asarray(gate_up_proj, dtype=np.float32)
    dp = np.asarray(down_proj, dtype=np.float32)

    t, h = x.shape
    e = gup.shape[0]
    assert (t, h, e) == (T, H, E), (t, h, e)

    # Per-(token, expert) combine weights; duplicate top-k slots merge.
    ar = np.arange(t)
    combine = np.zeros((t, e), np.float32)
    np.add.at(combine, (ar[:, None], idx), tkw)
    pres = np.zeros((t, e), bool)
    pres[ar[:, None], idx] = True
    tok_lists = [np.nonzero(pres[:, c])[0] for c in range(e)]

    cmax = max(1, max(len(tl) for tl in tok_lists))
    cap32 = ((cmax + 31) // 32) * 32
    nchunks = (cap32 + NMAX - 1) // NMAX
    nc_free = ((cap32 + nchunks - 1) // nchunks + 31) // 32 * 32
    cap = nc_free * nchunks

    nc = _get_program(nc_free, nchunks)

    in_maps = prepare_in_maps(x, gup, dp, tok_lists, nc_free, nchunks)

    res = run_bass_kernel_spmd(nc, in_maps, list(range(e)))

    out = np.zeros((t, h), np.float32)
    for c in range(e):
        tl = tok_lists[c]
        if not len(tl):
            continue
        yt = np.asarray(res.results[c]["yt"], np.float32).reshape(H, cap)
        out[tl] += combine[tl, c][:, None] * yt[:, :len(tl)].T
    return out
